# revision 8
# baseline (speedup 1.0000x reference)
"""Trainium2 Bass kernel for nn_Loss_20993800143146 (loss_fn).

Computes, over 8 NeuronCores (data-parallel over batch / bh):
    mel_loss  = mean(|mels_pred * mask - mels_target|)           (mean over full tensor)
    stop_loss = sum(-5 * clamp(log(stop_pred[b, last_idx_b]), -100)) / mask.sum()
    dc        = sum(alignments * band[s,t] * bmask[b]) / (H * lengths.sum() * N)
    out       = mel_loss + stop_loss - 1e-4 * dc

Key algebraic fact: band[s,t] = (s >= clip(5t-50,0,160)) & (s < clip(5t+50,0,160))
is identically zero for t >= 42 (clip hits s=160), so only alignments[:,:,:,:42]
is ever read (~5 MB of the 98 MB tensor).

Sharding: batch dim (16 -> 2 per core) for lengths/mask/stop/mels, bh dim
(64 -> 8 per core) for alignments. Each core reduces its shard to 8 partial
scalars on-device; the host sums the 8 partial vectors and applies the final
constant-denominator arithmetic.

Per-core layouts (self-contained; all shapes hardcoded):
  bigf [128, 3366] f32 = stop13S(13) | iota13S(13) | melsp(1040) | melst(1040)
                         | align(1260), DMA'd in 4 chunks on SP+ACT sequencers.
  - melsp/melst: [2,800,80] -> rows (b,t) padded 1600->1664 -> 13 rows of 80
    per partition (contiguous per partition).
  - align: alignments[:, 8c:8c+8, :, :42] transposed to [b_local, n, s, t],
    16 partitions per b_local, 30 (n,s)-rows of 42 per partition (contiguous).
  - stop13S/iota13S: stop_pred split per batch row: b0 -> partitions 0..63,
    b1 -> partitions 64..127, 13 t's per partition; iota holds t+1 (0 = pad).
  wband [128,1260] u8: band weight per align element; gpsimd cast-DMA -> f32.
  masks2 [128,26] u8: mask in mel layout (cols 0:13) and stop layout (13:26).
  lens [128,2] i32: col0 = lengths[b_local(p)] (bmask), col1 = lengths (p<16).

Stats tile [128,8] reduced across partitions with one PE matmul against ones:
  cols: 0=dc_w, 1=melA(sum m|d|), 2=melB(sum|b|), 3=melC(sum m|b|),
        4=mask_cnt, 5=sum clamped log p_last, 6=lengths_sum, 7=unused.
"""

import numpy as np

# Problem constants (hardcoded per contract; kernel.py must be self-contained).
H = 4
B = 16
T = 800
NMEL = 80
S = 160
N = 3
BW = 50
K = T // S  # 5
TC = 42  # band[:, t] == 0 for all t >= TC
NCORES = 8

MEL_ROWS = 2 * T            # 1600 (b,t) rows per core
MEL_PAD_ROWS = 1664         # pad to 128 * 13
MG = 13                     # 80-col groups per partition (mel) / t's (stop)
ALN_F = N * S * TC // 16    # 1260 free elems per partition (8 b * 16 part/b)
BIGF = 2 * MG + 2 * MG * NMEL + ALN_F  # 3366

_CACHE = {}


def _band():
    tr = np.arange(TC)
    mn = np.clip(K * tr - BW, 0, S)
    mx = np.clip(K * tr + BW, 0, S)
    rows = np.arange(S)
    return ((rows[:, None] >= mn[None, :]) & (rows[:, None] < mx[None, :]))


def _wband_u8():
    """Band weight tile [128, 1260]: partition p holds rows (p%16)*30+j of the
    (n, s) x t[:TC] block of one b; weight depends only on s = row % 160."""
    band = _band()  # [S, TC] bool
    p_idx = np.arange(128)
    j_idx = np.arange(30)
    s_of = (((p_idx[:, None] % 16) * 30) + j_idx[None, :]) % S  # [128, 30]
    return band[s_of].reshape(128, ALN_F).astype(np.uint8)


def _iota13s():
    """[128,13] f32: t+1 in the stop split layout, 0 in pad positions."""
    out = np.zeros((128, MG), np.float32)
    for p in range(128):
        base = 13 * (p % 64)
        for j in range(MG):
            t = base + j
            if t < T:
                out[p, j] = t + 1
    return out


def _split13(row, pad_value):
    """[800] -> [64,13] padded with pad_value."""
    out = np.full((64 * MG,), pad_value, row.dtype)
    out[:T] = row
    return out.reshape(64, MG)


def _build_bass():
    import concourse.bacc as bacc
    import concourse.tile as tile
    import concourse.mybir as mybir
    import concourse.bass_isa as bass_isa
    from contextlib import ExitStack

    f32 = mybir.dt.float32
    u8 = mybir.dt.uint8
    i32 = mybir.dt.int32
    Alu = mybir.AluOpType
    Act = mybir.ActivationFunctionType
    Ax = mybir.AxisListType
    Red = bass_isa.ReduceOp

    nc = bacc.Bacc("TRN2", target_bir_lowering=False, debug=False,
                   num_devices=NCORES)

    bigf = nc.dram_tensor("bigf", [128, BIGF], f32, kind="ExternalInput").ap()
    wband = nc.dram_tensor("wband", [128, ALN_F], u8, kind="ExternalInput").ap()
    masks2 = nc.dram_tensor("masks2", [128, 2 * MG], u8, kind="ExternalInput").ap()
    lens = nc.dram_tensor("lens", [128, 2], i32, kind="ExternalInput").ap()
    out = nc.dram_tensor("out", [8, 1], f32, kind="ExternalOutput").ap()

    C_STOP, C_IOTA = 0, MG
    C_MP = 2 * MG
    C_MT = C_MP + MG * NMEL
    C_AL = C_MT + MG * NMEL
    AL_HALF = ALN_F // 2  # 630

    with tile.TileContext(nc) as tc:
        with ExitStack() as ctx:
            pool = ctx.enter_context(tc.tile_pool(name="main", bufs=1))
            ppool = ctx.enter_context(tc.tile_pool(name="ps", bufs=1, space="PSUM"))

            big_t = pool.tile([128, BIGF], f32, tag="big")
            wf_t = pool.tile([128, ALN_F], f32, tag="wf")
            mk_t = pool.tile([128, 2 * MG], u8, tag="mk")
            ln_t = pool.tile([128, 2], i32, tag="ln")

            # ---- DMA issues (earliest-needed first) ----
            # SP sequencer: stop/iota chunk, then the two mel tensors.
            nc.sync.dma_start(big_t[:, 0:C_MP], bigf[:, 0:C_MP])
            # ACT sequencer: the tiny u8/i32 inputs, then align in 2 chunks.
            nc.scalar.dma_start(mk_t[:], masks2)
            nc.scalar.dma_start(ln_t[:], lens)
            # gpsimd SWDGE: band weights with u8 -> f32 cast during transfer.
            nc.gpsimd.dma_start(wf_t[:], wband)
            nc.sync.dma_start(big_t[:, C_MP:C_MT], bigf[:, C_MP:C_MT])
            nc.sync.dma_start(big_t[:, C_MT:C_AL], bigf[:, C_MT:C_AL])
            nc.scalar.dma_start(big_t[:, C_AL:C_AL + AL_HALF],
                                bigf[:, C_AL:C_AL + AL_HALF])
            nc.scalar.dma_start(big_t[:, C_AL + AL_HALF:BIGF],
                                bigf[:, C_AL + AL_HALF:BIGF])

            # stats[:, c]: 0=dc_w, 1=melA, 2=melB, 3=melC, 4=mask_cnt,
            # 5=logp, 6=len_sum, 7=pad
            st_t = pool.tile([128, 8], f32, tag="st")
            nc.vector.memset(st_t[:], 0.0)
            on_t = pool.tile([128, 1], f32, tag="on")
            nc.vector.memset(on_t[:], 1.0)

            stop_v = big_t[:, C_STOP:C_STOP + MG]
            iota_v = big_t[:, C_IOTA:C_IOTA + MG]
            mp_v = big_t[:, C_MP:C_MT].rearrange("p (g m) -> p g m", m=NMEL)
            mt_v = big_t[:, C_MT:C_AL].rearrange("p (g m) -> p g m", m=NMEL)

            # ---- stop term (b0 on partitions 0:64, b1 on 64:128) ----
            lp_t = pool.tile([128, MG], f32, tag="lp")
            nc.scalar.activation(lp_t[:], stop_v, Act.Ln)
            cl_t = pool.tile([128, MG], f32, tag="cl")
            nc.vector.tensor_scalar_max(cl_t[:], lp_t[:], -100.0)
            msf_t = pool.tile([128, MG], f32, tag="msf")
            nc.vector.tensor_copy(msf_t[:], mk_t[:, MG:2 * MG])
            tl_t = pool.tile([128, MG], f32, tag="tl")
            nc.vector.tensor_mul(tl_t[:], iota_v, msf_t[:])
            mxp_t = pool.tile([128, 1], f32, tag="mxp")
            nc.vector.tensor_reduce(mxp_t[:], tl_t[:], axis=Ax.X, op=Alu.max)
            mb_t = pool.tile([128, 1], f32, tag="mb")
            nc.gpsimd.partition_all_reduce(mb_t[0:64, 0:1], mxp_t[0:64, 0:1],
                                           channels=64, reduce_op=Red.max)
            nc.gpsimd.partition_all_reduce(mb_t[64:128, 0:1], mxp_t[64:128, 0:1],
                                           channels=64, reduce_op=Red.max)
            eqj_t = pool.tile([128, MG], f32, tag="eqj")
            nc.vector.scalar_tensor_tensor(
                eqj_t[:], tl_t[:], mb_t[:, 0:1], cl_t[:],
                op0=Alu.is_equal, op1=Alu.mult, accum_out=st_t[:, 5:6])

            # ---- mask count + lengths ----
            m13f_t = pool.tile([128, MG], f32, tag="m13f")
            nc.vector.tensor_copy(m13f_t[:], mk_t[:, 0:MG])
            nc.vector.tensor_reduce(st_t[:, 4:5], m13f_t[:], axis=Ax.X, op=Alu.add)
            lrf_t = pool.tile([128, 1], f32, tag="lrf")
            nc.vector.tensor_copy(lrf_t[:], ln_t[:, 0:1])
            nc.vector.tensor_copy(st_t[:, 6:7], ln_t[:, 1:2])
            bm_t = pool.tile([128, 1], f32, tag="bm")
            nc.vector.tensor_scalar(bm_t[:], lrf_t[:], float(T), None, op0=Alu.is_le)

            # ---- mel term ----
            d_t = pool.tile([128, MG * NMEL], f32, tag="d")
            nc.gpsimd.tensor_sub(d_t[:], mp_v, mt_v)
            v1_t = pool.tile([128, MG], f32, tag="v1")
            nc.vector.tensor_reduce(
                v1_t[:], d_t[:].rearrange("p (g m) -> p g m", m=NMEL),
                axis=Ax.X, op=Alu.add, apply_absolute_value=True)
            v2_t = pool.tile([128, MG], f32, tag="v2")
            nc.vector.tensor_reduce(v2_t[:], mt_v, axis=Ax.X, op=Alu.add,
                                    apply_absolute_value=True)
            w1_t = pool.tile([128, MG], f32, tag="w1")
            nc.vector.scalar_tensor_tensor(
                w1_t[:], v1_t[:], 1.0, m13f_t[:],
                op0=Alu.bypass, op1=Alu.mult, accum_out=st_t[:, 1:2])
            nc.vector.tensor_reduce(st_t[:, 2:3], v2_t[:], axis=Ax.X, op=Alu.add)
            w2_t = pool.tile([128, MG], f32, tag="w2")
            nc.vector.scalar_tensor_tensor(
                w2_t[:], v2_t[:], 1.0, m13f_t[:],
                op0=Alu.bypass, op1=Alu.mult, accum_out=st_t[:, 3:4])

            # ---- dc term (two halves so compute overlaps the 2nd DMA) ----
            al_v = big_t[:, C_AL:BIGF]
            pra_t = pool.tile([128, AL_HALF], f32, tag="pra")
            dca_t = pool.tile([128, 1], f32, tag="dca")
            nc.vector.scalar_tensor_tensor(
                pra_t[:], al_v[:, 0:AL_HALF], 1.0, wf_t[:, 0:AL_HALF],
                op0=Alu.bypass, op1=Alu.mult, accum_out=dca_t[:])
            prb_t = pool.tile([128, AL_HALF], f32, tag="prb")
            dcb_t = pool.tile([128, 1], f32, tag="dcb")
            nc.vector.scalar_tensor_tensor(
                prb_t[:], al_v[:, AL_HALF:ALN_F], 1.0, wf_t[:, AL_HALF:ALN_F],
                op0=Alu.bypass, op1=Alu.mult, accum_out=dcb_t[:])
            dcs_t = pool.tile([128, 1], f32, tag="dcs")
            nc.vector.tensor_add(dcs_t[:], dca_t[:], dcb_t[:])
            nc.vector.tensor_mul(st_t[:, 0:1], dcs_t[:], bm_t[:])

            # ---- partition reduction via PE: out[8,1] = stats.T @ ones ----
            pt = ppool.tile([8, 1], f32, tag="pt")
            nc.tensor.matmul(pt[:], lhsT=st_t[:], rhs=on_t[:],
                             start=True, stop=True)
            ex_t = pool.tile([8, 1], f32, tag="ex")
            nc.vector.tensor_copy(ex_t[:], pt[:])
            nc.sync.dma_start(out, ex_t[:])

    nc.compile()
    return nc


def _get_nc():
    if "nc" not in _CACHE:
        _CACHE["nc"] = _build_bass()
    return _CACHE["nc"]


def make_in_maps(lengths, mask, stop_pred, mels_pred, mels_target, alignments):
    """Shard full inputs into the 8 per-core input dicts."""
    lengths = np.ascontiguousarray(lengths, dtype=np.int32)
    mask_u8 = np.ascontiguousarray(mask).view(np.uint8) if mask.dtype == np.bool_ \
        else np.ascontiguousarray(mask.astype(np.uint8))
    stop_pred = np.ascontiguousarray(stop_pred, dtype=np.float32)
    mels_pred = np.ascontiguousarray(mels_pred, dtype=np.float32)
    mels_target = np.ascontiguousarray(mels_target, dtype=np.float32)
    alignments = np.ascontiguousarray(alignments, dtype=np.float32)

    wband = _wband_u8()
    iota13s = _iota13s()

    def pad_rows(x2d, cols):
        padded = np.zeros((MEL_PAD_ROWS, cols), x2d.dtype)
        padded[:MEL_ROWS] = x2d
        return padded

    in_maps = []
    for c in range(NCORES):
        bs = slice(2 * c, 2 * c + 2)
        bigf = np.empty((128, BIGF), np.float32)
        bigf[:, 0:MG] = np.concatenate(
            [_split13(stop_pred[2 * c], np.float32(1.0)),
             _split13(stop_pred[2 * c + 1], np.float32(1.0))])
        bigf[:, MG:2 * MG] = iota13s
        bigf[:, 2 * MG:2 * MG + MG * NMEL] = \
            pad_rows(mels_pred[bs].reshape(MEL_ROWS, NMEL), NMEL).reshape(128, MG * NMEL)
        bigf[:, 2 * MG + MG * NMEL:2 * MG + 2 * MG * NMEL] = \
            pad_rows(mels_target[bs].reshape(MEL_ROWS, NMEL), NMEL).reshape(128, MG * NMEL)
        bigf[:, 2 * MG + 2 * MG * NMEL:] = np.ascontiguousarray(
            alignments[:, 8 * c:8 * c + 8, :, :TC].transpose(1, 0, 2, 3)
        ).reshape(128, ALN_F)

        masks2 = np.zeros((128, 2 * MG), np.uint8)
        masks2[:, 0:MG] = pad_rows(mask_u8[bs].reshape(MEL_ROWS, 1), 1).reshape(128, MG)
        masks2[:, MG:2 * MG] = np.concatenate(
            [_split13(mask_u8[2 * c], np.uint8(0)),
             _split13(mask_u8[2 * c + 1], np.uint8(0))])

        b_lo = 8 * (c % 2)
        lens = np.zeros((128, 2), np.int32)
        lens[:, 0] = np.repeat(lengths[b_lo:b_lo + 8], 16)
        lens[:B, 1] = lengths

        in_maps.append({"bigf": bigf, "wband": wband, "masks2": masks2,
                        "lens": lens})
    return in_maps


def combine_partials(partials):
    """partials: list of 8 arrays [8,1] -> final scalar (0-d f32 ndarray)."""
    ps = np.stack([np.asarray(p, dtype=np.float64).reshape(8) for p in partials])
    dc_w = ps[:, 0].sum()
    mel_num = ps[:, 1].sum() + ps[:, 2].sum() - ps[:, 3].sum()
    logp = ps[:, 5].sum()
    mask_cnt = ps[:, 4].sum()
    len_sum = ps[0, 6]
    mel_loss = mel_num / float(B * T * NMEL)
    stop_loss = -5.0 * logp / mask_cnt
    dc = dc_w / (H * len_sum * N)
    return np.array(np.float32(mel_loss + stop_loss - 1e-4 * dc))


def kernel(lengths, mask, stop_pred, mels_pred, mels_target, alignments):
    from concourse.bass_utils import run_bass_kernel_spmd

    nc = _get_nc()
    in_maps = make_in_maps(lengths, np.asarray(mask), stop_pred,
                           mels_pred, mels_target, alignments)
    res = run_bass_kernel_spmd(nc, in_maps, list(range(NCORES)))
    return combine_partials([r["out"] for r in res.results])


# revision 9
# speedup vs baseline: 1.0103x; 1.0103x over previous
"""Trainium2 Bass kernel for nn_Loss_20993800143146 (loss_fn).

Computes, over 8 NeuronCores (data-parallel over batch / bh):
    mel_loss  = mean(|mels_pred * mask - mels_target|)           (mean over full tensor)
    stop_loss = sum(-5 * clamp(log(stop_pred[b, last_idx_b]), -100)) / mask.sum()
    dc        = sum(alignments * band[s,t] * bmask[b]) / (H * lengths.sum() * N)
    out       = mel_loss + stop_loss - 1e-4 * dc

Key algebraic fact: band[s,t] = (s >= clip(5t-50,0,160)) & (s < clip(5t+50,0,160))
is identically zero for t >= 42 (clip hits s=160), so only alignments[:,:,:,:42]
is ever read (~5 MB of the 98 MB tensor).

Sharding: batch dim (16 -> 2 per core) for lengths/mask/stop/mels, bh dim
(64 -> 8 per core) for alignments. Each core reduces its shard to 8 partial
scalars on-device; the host sums the 8 partial vectors and applies the final
constant-denominator arithmetic.

Per-core layouts (self-contained; all shapes hardcoded):
  bigf [128, 3366] f32 = stop13S(13) | iota13S(13) | melsp(1040) | melst(1040)
                         | align(1260), DMA'd in 4 chunks on SP+ACT sequencers.
  - melsp/melst: [2,800,80] -> rows (b,t) padded 1600->1664 -> 13 rows of 80
    per partition (contiguous per partition).
  - align: alignments[:, 8c:8c+8, :, :42] transposed to [b_local, n, s, t],
    16 partitions per b_local, 30 (n,s)-rows of 42 per partition (contiguous).
  - stop13S/iota13S: stop_pred split per batch row: b0 -> partitions 0..63,
    b1 -> partitions 64..127, 13 t's per partition; iota holds t+1 (0 = pad).
  wband [128,1260] u8: band weight per align element; gpsimd cast-DMA -> f32.
  masks2 [128,26] u8: mask in mel layout (cols 0:13) and stop layout (13:26).
  lens [128,2] i32: col0 = lengths[b_local(p)] (bmask), col1 = lengths (p<16).

Stats tile [128,8] reduced across partitions with one PE matmul against ones:
  cols: 0=dc_w, 1=melA(sum m|d|), 2=melB(sum|b|), 3=melC(sum m|b|),
        4=mask_cnt, 5=sum clamped log p_last, 6=lengths_sum, 7=unused.
"""

import numpy as np

# Problem constants (hardcoded per contract; kernel.py must be self-contained).
H = 4
B = 16
T = 800
NMEL = 80
S = 160
N = 3
BW = 50
K = T // S  # 5
TC = 42  # band[:, t] == 0 for all t >= TC
NCORES = 8

MEL_ROWS = 2 * T            # 1600 (b,t) rows per core
MEL_PAD_ROWS = 1664         # pad to 128 * 13
MG = 13                     # 80-col groups per partition (mel) / t's (stop)
ALN_F = N * S * TC // 16    # 1260 free elems per partition (8 b * 16 part/b)
BIGF = 2 * MG + 2 * MG * NMEL + ALN_F  # 3366

_CACHE = {}


def _band():
    tr = np.arange(TC)
    mn = np.clip(K * tr - BW, 0, S)
    mx = np.clip(K * tr + BW, 0, S)
    rows = np.arange(S)
    return ((rows[:, None] >= mn[None, :]) & (rows[:, None] < mx[None, :]))


def _wband_u8():
    """Band weight tile [128, 1260]: partition p holds rows (p%16)*30+j of the
    (n, s) x t[:TC] block of one b; weight depends only on s = row % 160."""
    band = _band()  # [S, TC] bool
    p_idx = np.arange(128)
    j_idx = np.arange(30)
    s_of = (((p_idx[:, None] % 16) * 30) + j_idx[None, :]) % S  # [128, 30]
    return band[s_of].reshape(128, ALN_F).astype(np.uint8)


def _iota13s():
    """[128,13] f32: t+1 in the stop split layout, 0 in pad positions."""
    out = np.zeros((128, MG), np.float32)
    for p in range(128):
        base = 13 * (p % 64)
        for j in range(MG):
            t = base + j
            if t < T:
                out[p, j] = t + 1
    return out


def _split13(row, pad_value):
    """[800] -> [64,13] padded with pad_value."""
    out = np.full((64 * MG,), pad_value, row.dtype)
    out[:T] = row
    return out.reshape(64, MG)


def _build_bass():
    import concourse.bacc as bacc
    import concourse.tile as tile
    import concourse.mybir as mybir
    import concourse.bass_isa as bass_isa
    from contextlib import ExitStack

    f32 = mybir.dt.float32
    u8 = mybir.dt.uint8
    i32 = mybir.dt.int32
    Alu = mybir.AluOpType
    Act = mybir.ActivationFunctionType
    Ax = mybir.AxisListType
    Red = bass_isa.ReduceOp

    nc = bacc.Bacc("TRN2", target_bir_lowering=False, debug=False,
                   num_devices=NCORES)

    bigf = nc.dram_tensor("bigf", [128, BIGF], f32, kind="ExternalInput").ap()
    wband = nc.dram_tensor("wband", [128, ALN_F], u8, kind="ExternalInput").ap()
    masks2 = nc.dram_tensor("masks2", [128, 2 * MG], u8, kind="ExternalInput").ap()
    lens = nc.dram_tensor("lens", [128, 2], i32, kind="ExternalInput").ap()
    out = nc.dram_tensor("out", [8, 1], f32, kind="ExternalOutput").ap()

    C_STOP, C_IOTA = 0, MG
    C_MP = 2 * MG
    C_MT = C_MP + MG * NMEL
    C_AL = C_MT + MG * NMEL
    AL_HALF = ALN_F // 2  # 630

    with tile.TileContext(nc) as tc:
        with ExitStack() as ctx:
            pool = ctx.enter_context(tc.tile_pool(name="main", bufs=1))
            ppool = ctx.enter_context(tc.tile_pool(name="ps", bufs=1, space="PSUM"))

            big_t = pool.tile([128, BIGF], f32, tag="big")
            wf_t = pool.tile([128, ALN_F], f32, tag="wf")
            mk_t = pool.tile([128, 2 * MG], u8, tag="mk")
            ln_t = pool.tile([128, 2], i32, tag="ln")

            # ---- DMA issues (earliest-needed first) ----
            # SP sequencer: stop/iota chunk, then the two mel tensors.
            nc.sync.dma_start(big_t[:, 0:C_MP], bigf[:, 0:C_MP])
            # ACT sequencer: the tiny u8/i32 inputs, then align in 2 chunks.
            nc.scalar.dma_start(mk_t[:], masks2)
            nc.scalar.dma_start(ln_t[:], lens)
            # gpsimd SWDGE: band weights with u8 -> f32 cast during transfer.
            nc.gpsimd.dma_start(wf_t[:], wband)
            nc.sync.dma_start(big_t[:, C_MP:C_MT], bigf[:, C_MP:C_MT])
            nc.sync.dma_start(big_t[:, C_MT:C_AL], bigf[:, C_MT:C_AL])
            nc.scalar.dma_start(big_t[:, C_AL:C_AL + AL_HALF],
                                bigf[:, C_AL:C_AL + AL_HALF])
            nc.scalar.dma_start(big_t[:, C_AL + AL_HALF:BIGF],
                                bigf[:, C_AL + AL_HALF:BIGF])

            # stats[:, c]: 0=dc_w, 1=melA, 2=melB, 3=melC, 4=mask_cnt,
            # 5=logp, 6=len_sum, 7=pad
            st_t = pool.tile([128, 8], f32, tag="st")
            nc.vector.memset(st_t[:], 0.0)
            on_t = pool.tile([128, 1], f32, tag="on")
            nc.vector.memset(on_t[:], 1.0)

            stop_v = big_t[:, C_STOP:C_STOP + MG]
            iota_v = big_t[:, C_IOTA:C_IOTA + MG]
            mp_v = big_t[:, C_MP:C_MT].rearrange("p (g m) -> p g m", m=NMEL)
            mt_v = big_t[:, C_MT:C_AL].rearrange("p (g m) -> p g m", m=NMEL)

            # ---- stop term (b0 on partitions 0:64, b1 on 64:128) ----
            lp_t = pool.tile([128, MG], f32, tag="lp")
            nc.scalar.activation(lp_t[:], stop_v, Act.Ln)
            cl_t = pool.tile([128, MG], f32, tag="cl")
            nc.vector.tensor_scalar_max(cl_t[:], lp_t[:], -100.0)
            msf_t = pool.tile([128, MG], f32, tag="msf")
            nc.vector.tensor_copy(msf_t[:], mk_t[:, MG:2 * MG])
            tl_t = pool.tile([128, MG], f32, tag="tl")
            nc.vector.tensor_mul(tl_t[:], iota_v, msf_t[:])
            # Per-partition max, then per-b max across partitions. HW
            # partition_all_reduce ignores AP partition offsets, so run ONE
            # full-128 call on a 2-column tile (col0 = b0, col1 = b1).
            mx2_t = pool.tile([128, 2], f32, tag="mx2")
            nc.vector.memset(mx2_t[:], 0.0)
            nc.vector.tensor_reduce(mx2_t[0:64, 0:1], tl_t[0:64, :],
                                    axis=Ax.X, op=Alu.max)
            nc.vector.tensor_reduce(mx2_t[64:128, 1:2], tl_t[64:128, :],
                                    axis=Ax.X, op=Alu.max)
            mb2_t = pool.tile([128, 2], f32, tag="mb2")
            nc.gpsimd.partition_all_reduce(mb2_t[:], mx2_t[:],
                                           channels=128, reduce_op=Red.max)
            eqj_t = pool.tile([128, MG], f32, tag="eqj")
            nc.vector.scalar_tensor_tensor(
                eqj_t[0:64, :], tl_t[0:64, :], mb2_t[0:64, 0:1], cl_t[0:64, :],
                op0=Alu.is_equal, op1=Alu.mult, accum_out=st_t[0:64, 5:6])
            nc.vector.scalar_tensor_tensor(
                eqj_t[64:128, :], tl_t[64:128, :], mb2_t[64:128, 1:2],
                cl_t[64:128, :],
                op0=Alu.is_equal, op1=Alu.mult, accum_out=st_t[64:128, 5:6])

            # ---- mask count + lengths ----
            m13f_t = pool.tile([128, MG], f32, tag="m13f")
            nc.vector.tensor_copy(m13f_t[:], mk_t[:, 0:MG])
            nc.vector.tensor_reduce(st_t[:, 4:5], m13f_t[:], axis=Ax.X, op=Alu.add)
            lrf_t = pool.tile([128, 1], f32, tag="lrf")
            nc.vector.tensor_copy(lrf_t[:], ln_t[:, 0:1])
            nc.vector.tensor_copy(st_t[:, 6:7], ln_t[:, 1:2])
            bm_t = pool.tile([128, 1], f32, tag="bm")
            nc.vector.tensor_scalar(bm_t[:], lrf_t[:], float(T), None, op0=Alu.is_le)

            # ---- mel term ----
            d_t = pool.tile([128, MG * NMEL], f32, tag="d")
            nc.gpsimd.tensor_sub(d_t[:], mp_v, mt_v)
            v1_t = pool.tile([128, MG], f32, tag="v1")
            nc.vector.tensor_reduce(
                v1_t[:], d_t[:].rearrange("p (g m) -> p g m", m=NMEL),
                axis=Ax.X, op=Alu.add, apply_absolute_value=True)
            v2_t = pool.tile([128, MG], f32, tag="v2")
            nc.vector.tensor_reduce(v2_t[:], mt_v, axis=Ax.X, op=Alu.add,
                                    apply_absolute_value=True)
            w1_t = pool.tile([128, MG], f32, tag="w1")
            nc.vector.scalar_tensor_tensor(
                w1_t[:], v1_t[:], 1.0, m13f_t[:],
                op0=Alu.bypass, op1=Alu.mult, accum_out=st_t[:, 1:2])
            nc.vector.tensor_reduce(st_t[:, 2:3], v2_t[:], axis=Ax.X, op=Alu.add)
            w2_t = pool.tile([128, MG], f32, tag="w2")
            nc.vector.scalar_tensor_tensor(
                w2_t[:], v2_t[:], 1.0, m13f_t[:],
                op0=Alu.bypass, op1=Alu.mult, accum_out=st_t[:, 3:4])

            # ---- dc term (two halves so compute overlaps the 2nd DMA) ----
            al_v = big_t[:, C_AL:BIGF]
            pra_t = pool.tile([128, AL_HALF], f32, tag="pra")
            dca_t = pool.tile([128, 1], f32, tag="dca")
            nc.vector.scalar_tensor_tensor(
                pra_t[:], al_v[:, 0:AL_HALF], 1.0, wf_t[:, 0:AL_HALF],
                op0=Alu.bypass, op1=Alu.mult, accum_out=dca_t[:])
            prb_t = pool.tile([128, AL_HALF], f32, tag="prb")
            dcb_t = pool.tile([128, 1], f32, tag="dcb")
            nc.vector.scalar_tensor_tensor(
                prb_t[:], al_v[:, AL_HALF:ALN_F], 1.0, wf_t[:, AL_HALF:ALN_F],
                op0=Alu.bypass, op1=Alu.mult, accum_out=dcb_t[:])
            dcs_t = pool.tile([128, 1], f32, tag="dcs")
            nc.vector.tensor_add(dcs_t[:], dca_t[:], dcb_t[:])
            nc.vector.tensor_mul(st_t[:, 0:1], dcs_t[:], bm_t[:])

            # ---- partition reduction via PE: out[8,1] = stats.T @ ones ----
            pt = ppool.tile([8, 1], f32, tag="pt")
            nc.tensor.matmul(pt[:], lhsT=st_t[:], rhs=on_t[:],
                             start=True, stop=True)
            ex_t = pool.tile([8, 1], f32, tag="ex")
            nc.vector.tensor_copy(ex_t[:], pt[:])
            nc.sync.dma_start(out, ex_t[:])

    nc.compile()
    return nc


def _get_nc():
    if "nc" not in _CACHE:
        _CACHE["nc"] = _build_bass()
    return _CACHE["nc"]


def make_in_maps(lengths, mask, stop_pred, mels_pred, mels_target, alignments):
    """Shard full inputs into the 8 per-core input dicts."""
    lengths = np.ascontiguousarray(lengths, dtype=np.int32)
    mask_u8 = np.ascontiguousarray(mask).view(np.uint8) if mask.dtype == np.bool_ \
        else np.ascontiguousarray(mask.astype(np.uint8))
    stop_pred = np.ascontiguousarray(stop_pred, dtype=np.float32)
    mels_pred = np.ascontiguousarray(mels_pred, dtype=np.float32)
    mels_target = np.ascontiguousarray(mels_target, dtype=np.float32)
    alignments = np.ascontiguousarray(alignments, dtype=np.float32)

    wband = _wband_u8()
    iota13s = _iota13s()

    def pad_rows(x2d, cols):
        padded = np.zeros((MEL_PAD_ROWS, cols), x2d.dtype)
        padded[:MEL_ROWS] = x2d
        return padded

    in_maps = []
    for c in range(NCORES):
        bs = slice(2 * c, 2 * c + 2)
        bigf = np.empty((128, BIGF), np.float32)
        bigf[:, 0:MG] = np.concatenate(
            [_split13(stop_pred[2 * c], np.float32(1.0)),
             _split13(stop_pred[2 * c + 1], np.float32(1.0))])
        bigf[:, MG:2 * MG] = iota13s
        bigf[:, 2 * MG:2 * MG + MG * NMEL] = \
            pad_rows(mels_pred[bs].reshape(MEL_ROWS, NMEL), NMEL).reshape(128, MG * NMEL)
        bigf[:, 2 * MG + MG * NMEL:2 * MG + 2 * MG * NMEL] = \
            pad_rows(mels_target[bs].reshape(MEL_ROWS, NMEL), NMEL).reshape(128, MG * NMEL)
        bigf[:, 2 * MG + 2 * MG * NMEL:] = np.ascontiguousarray(
            alignments[:, 8 * c:8 * c + 8, :, :TC].transpose(1, 0, 2, 3)
        ).reshape(128, ALN_F)

        masks2 = np.zeros((128, 2 * MG), np.uint8)
        masks2[:, 0:MG] = pad_rows(mask_u8[bs].reshape(MEL_ROWS, 1), 1).reshape(128, MG)
        masks2[:, MG:2 * MG] = np.concatenate(
            [_split13(mask_u8[2 * c], np.uint8(0)),
             _split13(mask_u8[2 * c + 1], np.uint8(0))])

        b_lo = 8 * (c % 2)
        lens = np.zeros((128, 2), np.int32)
        lens[:, 0] = np.repeat(lengths[b_lo:b_lo + 8], 16)
        lens[:B, 1] = lengths

        in_maps.append({"bigf": bigf, "wband": wband, "masks2": masks2,
                        "lens": lens})
    return in_maps


def combine_partials(partials):
    """partials: list of 8 arrays [8,1] -> final scalar (0-d f32 ndarray)."""
    ps = np.stack([np.asarray(p, dtype=np.float64).reshape(8) for p in partials])
    dc_w = ps[:, 0].sum()
    mel_num = ps[:, 1].sum() + ps[:, 2].sum() - ps[:, 3].sum()
    logp = ps[:, 5].sum()
    mask_cnt = ps[:, 4].sum()
    len_sum = ps[0, 6]
    mel_loss = mel_num / float(B * T * NMEL)
    stop_loss = -5.0 * logp / mask_cnt
    dc = dc_w / (H * len_sum * N)
    return np.array(np.float32(mel_loss + stop_loss - 1e-4 * dc))


def kernel(lengths, mask, stop_pred, mels_pred, mels_target, alignments):
    from concourse.bass_utils import run_bass_kernel_spmd

    nc = _get_nc()
    in_maps = make_in_maps(lengths, np.asarray(mask), stop_pred,
                           mels_pred, mels_target, alignments)
    res = run_bass_kernel_spmd(nc, in_maps, list(range(NCORES)))
    return combine_partials([r["out"] for r in res.results])


# revision 11
# speedup vs baseline: 1.1087x; 1.0974x over previous
"""Trainium2 Bass kernel for nn_Loss_20993800143146 (loss_fn).

Computes, over 8 NeuronCores (data-parallel over batch / bh):
    mel_loss  = mean(|mels_pred * mask - mels_target|)           (mean over full tensor)
    stop_loss = sum(-5 * clamp(log(stop_pred[b, last_idx_b]), -100)) / mask.sum()
    dc        = sum(alignments * band[s,t] * bmask[b]) / (H * lengths.sum() * N)
    out       = mel_loss + stop_loss - 1e-4 * dc

Key algebraic fact: band[s,t] = (s >= clip(5t-50,0,160)) & (s < clip(5t+50,0,160))
is identically zero for t >= 42 (clip hits s=160), so only alignments[:,:,:,:42]
is ever read (~5 MB of the 98 MB tensor).

Sharding: batch dim (16 -> 2 per core) for lengths/mask/stop/mels, bh dim
(64 -> 8 per core) for alignments. Each core reduces its shard to 8 partial
scalars on-device; the host sums the 8 partial vectors and applies the final
constant-denominator arithmetic.

Per-core layouts (self-contained; all shapes hardcoded):
  bigf [128, 3366] f32 = stop13S(13) | iota13S(13) | melsp(1040) | melst(1040)
                         | align(1260), DMA'd in 4 chunks on SP+ACT sequencers.
  - melsp/melst: [2,800,80] -> rows (b,t) padded 1600->1664 -> 13 rows of 80
    per partition (contiguous per partition).
  - align: alignments[:, 8c:8c+8, :, :42] transposed to [b_local, n, s, t],
    16 partitions per b_local, 30 (n,s)-rows of 42 per partition (contiguous).
  - stop13S/iota13S: stop_pred split per batch row: b0 -> partitions 0..63,
    b1 -> partitions 64..127, 13 t's per partition; iota holds t+1 (0 = pad).
  wband [128,1260] u8: band weight per align element; gpsimd cast-DMA -> f32.
  masks2 [128,26] u8: mask in mel layout (cols 0:13) and stop layout (13:26).
  lens [128,2] i32: col0 = lengths[b_local(p)] (bmask), col1 = lengths (p<16).

Stats tile [128,8] reduced across partitions with one PE matmul against ones:
  cols: 0=dc_w, 1=melA(sum m|d|), 2=melB(sum|b|), 3=melC(sum m|b|),
        4=mask_cnt, 5=sum clamped log p_last, 6=lengths_sum, 7=unused.
"""

import numpy as np

# Problem constants (hardcoded per contract; kernel.py must be self-contained).
H = 4
B = 16
T = 800
NMEL = 80
S = 160
N = 3
BW = 50
K = T // S  # 5
TC = 42  # band[:, t] == 0 for all t >= TC
NCORES = 8

MEL_ROWS = 2 * T            # 1600 (b,t) rows per core
MEL_PAD_ROWS = 1664         # pad to 128 * 13
MG = 13                     # 80-col groups per partition (mel) / t's (stop)
ALN_F = N * S * TC // 16    # 1260 free elems per partition (8 b * 16 part/b)
BIGF = 2 * MG + 2 * MG * NMEL + ALN_F  # 3366

_CACHE = {}


def _band():
    tr = np.arange(TC)
    mn = np.clip(K * tr - BW, 0, S)
    mx = np.clip(K * tr + BW, 0, S)
    rows = np.arange(S)
    return ((rows[:, None] >= mn[None, :]) & (rows[:, None] < mx[None, :]))


def _wband_u8():
    """Band weight tile [128, 1260]: partition p holds rows (p%16)*30+j of the
    (n, s) x t[:TC] block of one b; weight depends only on s = row % 160."""
    band = _band()  # [S, TC] bool
    p_idx = np.arange(128)
    j_idx = np.arange(30)
    s_of = (((p_idx[:, None] % 16) * 30) + j_idx[None, :]) % S  # [128, 30]
    return band[s_of].reshape(128, ALN_F).astype(np.uint8)


def _iota13s():
    """[128,13] f32: t+1 in the stop split layout, 0 in pad positions."""
    out = np.zeros((128, MG), np.float32)
    for p in range(128):
        base = 13 * (p % 64)
        for j in range(MG):
            t = base + j
            if t < T:
                out[p, j] = t + 1
    return out


def _split13(row, pad_value):
    """[800] -> [64,13] padded with pad_value."""
    out = np.full((64 * MG,), pad_value, row.dtype)
    out[:T] = row
    return out.reshape(64, MG)


def _build_bass():
    import concourse.bacc as bacc
    import concourse.tile as tile
    import concourse.mybir as mybir
    import concourse.bass_isa as bass_isa
    from contextlib import ExitStack

    f32 = mybir.dt.float32
    u8 = mybir.dt.uint8
    i32 = mybir.dt.int32
    Alu = mybir.AluOpType
    Act = mybir.ActivationFunctionType
    Ax = mybir.AxisListType
    Red = bass_isa.ReduceOp

    nc = bacc.Bacc("TRN2", target_bir_lowering=False, debug=False,
                   num_devices=NCORES)

    bigf = nc.dram_tensor("bigf", [128, BIGF], f32, kind="ExternalInput").ap()
    wband = nc.dram_tensor("wband", [128, ALN_F], u8, kind="ExternalInput").ap()
    masks2 = nc.dram_tensor("masks2", [128, 2 * MG], u8, kind="ExternalInput").ap()
    lens = nc.dram_tensor("lens", [128, 2], i32, kind="ExternalInput").ap()
    out = nc.dram_tensor("out", [8, 1], f32, kind="ExternalOutput").ap()

    C_STOP, C_IOTA = 0, MG
    C_MP = 2 * MG
    C_MT = C_MP + MG * NMEL
    C_AL = C_MT + MG * NMEL
    AL_HALF = ALN_F // 2  # 630

    with tile.TileContext(nc) as tc:
        with ExitStack() as ctx:
            pool = ctx.enter_context(tc.tile_pool(name="main", bufs=1))
            ppool = ctx.enter_context(tc.tile_pool(name="ps", bufs=1, space="PSUM"))

            big_t = pool.tile([128, BIGF], f32, tag="big")
            wf_t = pool.tile([128, ALN_F], f32, tag="wf")
            mk_t = pool.tile([128, 2 * MG], u8, tag="mk")
            ln_t = pool.tile([128, 2], i32, tag="ln")

            # ---- DMA issues (earliest-needed first) ----
            wb_t = pool.tile([128, ALN_F], u8, tag="wb")
            # SP sequencer: stop/iota chunk, band weights, the two mel tensors.
            nc.sync.dma_start(big_t[:, 0:C_MP], bigf[:, 0:C_MP])
            # ACT sequencer: the tiny u8/i32 inputs, then align in 2 chunks.
            nc.scalar.dma_start(mk_t[:], masks2)
            nc.scalar.dma_start(ln_t[:], lens)
            nc.sync.dma_start(wb_t[:], wband)
            nc.sync.dma_start(big_t[:, C_MP:C_MT], bigf[:, C_MP:C_MT])
            nc.sync.dma_start(big_t[:, C_MT:C_AL], bigf[:, C_MT:C_AL])
            nc.scalar.dma_start(big_t[:, C_AL:C_AL + AL_HALF],
                                bigf[:, C_AL:C_AL + AL_HALF])
            nc.scalar.dma_start(big_t[:, C_AL + AL_HALF:BIGF],
                                bigf[:, C_AL + AL_HALF:BIGF])
            # band-weight u8 -> f32 cast on the (otherwise idle) gpsimd engine
            nc.gpsimd.tensor_copy(wf_t[:], wb_t[:])

            # stats[:, c]: 0=dc_w, 1=melA, 2=melB, 3=melC, 4=mask_cnt,
            # 5=logp, 6=len_sum, 7=pad
            st_t = pool.tile([128, 8], f32, tag="st")
            nc.vector.memset(st_t[:], 0.0)
            on_t = pool.tile([128, 1], f32, tag="on")
            nc.vector.memset(on_t[:], 1.0)

            stop_v = big_t[:, C_STOP:C_STOP + MG]
            iota_v = big_t[:, C_IOTA:C_IOTA + MG]
            mp_v = big_t[:, C_MP:C_MT].rearrange("p (g m) -> p g m", m=NMEL)
            mt_v = big_t[:, C_MT:C_AL].rearrange("p (g m) -> p g m", m=NMEL)

            # ---- stop term (b0 on partitions 0:64, b1 on 64:128) ----
            lp_t = pool.tile([128, MG], f32, tag="lp")
            nc.scalar.activation(lp_t[:], stop_v, Act.Ln)
            cl_t = pool.tile([128, MG], f32, tag="cl")
            nc.vector.tensor_scalar_max(cl_t[:], lp_t[:], -100.0)
            msf_t = pool.tile([128, MG], f32, tag="msf")
            nc.vector.tensor_copy(msf_t[:], mk_t[:, MG:2 * MG])
            tl_t = pool.tile([128, MG], f32, tag="tl")
            nc.vector.tensor_mul(tl_t[:], iota_v, msf_t[:])
            # Per-partition max, then per-b max across partitions. HW
            # partition_all_reduce ignores AP partition offsets, so run ONE
            # full-128 call on a 2-column tile (col0 = b0, col1 = b1).
            mx2_t = pool.tile([128, 2], f32, tag="mx2")
            nc.vector.memset(mx2_t[:], 0.0)
            nc.vector.tensor_reduce(mx2_t[0:64, 0:1], tl_t[0:64, :],
                                    axis=Ax.X, op=Alu.max)
            nc.vector.tensor_reduce(mx2_t[64:128, 1:2], tl_t[64:128, :],
                                    axis=Ax.X, op=Alu.max)
            mb2_t = pool.tile([128, 2], f32, tag="mb2")
            nc.gpsimd.partition_all_reduce(mb2_t[:], mx2_t[:],
                                           channels=128, reduce_op=Red.max)
            eqj_t = pool.tile([128, MG], f32, tag="eqj")
            nc.vector.scalar_tensor_tensor(
                eqj_t[0:64, :], tl_t[0:64, :], mb2_t[0:64, 0:1], cl_t[0:64, :],
                op0=Alu.is_equal, op1=Alu.mult, accum_out=st_t[0:64, 5:6])
            nc.vector.scalar_tensor_tensor(
                eqj_t[64:128, :], tl_t[64:128, :], mb2_t[64:128, 1:2],
                cl_t[64:128, :],
                op0=Alu.is_equal, op1=Alu.mult, accum_out=st_t[64:128, 5:6])

            # ---- mask count + lengths ----
            m13f_t = pool.tile([128, MG], f32, tag="m13f")
            nc.vector.tensor_copy(m13f_t[:], mk_t[:, 0:MG])
            nc.vector.tensor_reduce(st_t[:, 4:5], m13f_t[:], axis=Ax.X, op=Alu.add)
            lrf_t = pool.tile([128, 1], f32, tag="lrf")
            nc.vector.tensor_copy(lrf_t[:], ln_t[:, 0:1])
            nc.vector.tensor_copy(st_t[:, 6:7], ln_t[:, 1:2])
            bm_t = pool.tile([128, 1], f32, tag="bm")
            nc.vector.tensor_scalar(bm_t[:], lrf_t[:], float(T), None, op0=Alu.is_le)

            # ---- mel term ----
            d_t = pool.tile([128, MG * NMEL], f32, tag="d")
            nc.vector.tensor_sub(d_t[:], mp_v, mt_v)
            v1_t = pool.tile([128, MG], f32, tag="v1")
            nc.vector.tensor_reduce(
                v1_t[:], d_t[:].rearrange("p (g m) -> p g m", m=NMEL),
                axis=Ax.X, op=Alu.add, apply_absolute_value=True)
            v2_t = pool.tile([128, MG], f32, tag="v2")
            nc.vector.tensor_reduce(v2_t[:], mt_v, axis=Ax.X, op=Alu.add,
                                    apply_absolute_value=True)
            w1_t = pool.tile([128, MG], f32, tag="w1")
            nc.vector.scalar_tensor_tensor(
                w1_t[:], v1_t[:], 1.0, m13f_t[:],
                op0=Alu.bypass, op1=Alu.mult, accum_out=st_t[:, 1:2])
            nc.vector.tensor_reduce(st_t[:, 2:3], v2_t[:], axis=Ax.X, op=Alu.add)
            w2_t = pool.tile([128, MG], f32, tag="w2")
            nc.vector.scalar_tensor_tensor(
                w2_t[:], v2_t[:], 1.0, m13f_t[:],
                op0=Alu.bypass, op1=Alu.mult, accum_out=st_t[:, 3:4])

            # ---- dc term (two halves so compute overlaps the 2nd DMA) ----
            al_v = big_t[:, C_AL:BIGF]
            pra_t = pool.tile([128, AL_HALF], f32, tag="pra")
            dca_t = pool.tile([128, 1], f32, tag="dca")
            nc.vector.scalar_tensor_tensor(
                pra_t[:], al_v[:, 0:AL_HALF], 1.0, wf_t[:, 0:AL_HALF],
                op0=Alu.bypass, op1=Alu.mult, accum_out=dca_t[:])
            prb_t = pool.tile([128, AL_HALF], f32, tag="prb")
            dcb_t = pool.tile([128, 1], f32, tag="dcb")
            nc.vector.scalar_tensor_tensor(
                prb_t[:], al_v[:, AL_HALF:ALN_F], 1.0, wf_t[:, AL_HALF:ALN_F],
                op0=Alu.bypass, op1=Alu.mult, accum_out=dcb_t[:])
            dcs_t = pool.tile([128, 1], f32, tag="dcs")
            nc.vector.tensor_add(dcs_t[:], dca_t[:], dcb_t[:])
            nc.vector.tensor_mul(st_t[:, 0:1], dcs_t[:], bm_t[:])

            # ---- partition reduction via PE: out[8,1] = stats.T @ ones ----
            pt = ppool.tile([8, 1], f32, tag="pt")
            nc.tensor.matmul(pt[:], lhsT=st_t[:], rhs=on_t[:],
                             start=True, stop=True)
            ex_t = pool.tile([8, 1], f32, tag="ex")
            nc.vector.tensor_copy(ex_t[:], pt[:])
            nc.sync.dma_start(out, ex_t[:])

    nc.compile()
    return nc


def _get_nc():
    if "nc" not in _CACHE:
        _CACHE["nc"] = _build_bass()
    return _CACHE["nc"]


def make_in_maps(lengths, mask, stop_pred, mels_pred, mels_target, alignments):
    """Shard full inputs into the 8 per-core input dicts."""
    lengths = np.ascontiguousarray(lengths, dtype=np.int32)
    mask_u8 = np.ascontiguousarray(mask).view(np.uint8) if mask.dtype == np.bool_ \
        else np.ascontiguousarray(mask.astype(np.uint8))
    stop_pred = np.ascontiguousarray(stop_pred, dtype=np.float32)
    mels_pred = np.ascontiguousarray(mels_pred, dtype=np.float32)
    mels_target = np.ascontiguousarray(mels_target, dtype=np.float32)
    alignments = np.ascontiguousarray(alignments, dtype=np.float32)

    wband = _wband_u8()
    iota13s = _iota13s()

    def pad_rows(x2d, cols):
        padded = np.zeros((MEL_PAD_ROWS, cols), x2d.dtype)
        padded[:MEL_ROWS] = x2d
        return padded

    in_maps = []
    for c in range(NCORES):
        bs = slice(2 * c, 2 * c + 2)
        bigf = np.empty((128, BIGF), np.float32)
        bigf[:, 0:MG] = np.concatenate(
            [_split13(stop_pred[2 * c], np.float32(1.0)),
             _split13(stop_pred[2 * c + 1], np.float32(1.0))])
        bigf[:, MG:2 * MG] = iota13s
        bigf[:, 2 * MG:2 * MG + MG * NMEL] = \
            pad_rows(mels_pred[bs].reshape(MEL_ROWS, NMEL), NMEL).reshape(128, MG * NMEL)
        bigf[:, 2 * MG + MG * NMEL:2 * MG + 2 * MG * NMEL] = \
            pad_rows(mels_target[bs].reshape(MEL_ROWS, NMEL), NMEL).reshape(128, MG * NMEL)
        bigf[:, 2 * MG + 2 * MG * NMEL:] = np.ascontiguousarray(
            alignments[:, 8 * c:8 * c + 8, :, :TC].transpose(1, 0, 2, 3)
        ).reshape(128, ALN_F)

        masks2 = np.zeros((128, 2 * MG), np.uint8)
        masks2[:, 0:MG] = pad_rows(mask_u8[bs].reshape(MEL_ROWS, 1), 1).reshape(128, MG)
        masks2[:, MG:2 * MG] = np.concatenate(
            [_split13(mask_u8[2 * c], np.uint8(0)),
             _split13(mask_u8[2 * c + 1], np.uint8(0))])

        b_lo = 8 * (c % 2)
        lens = np.zeros((128, 2), np.int32)
        lens[:, 0] = np.repeat(lengths[b_lo:b_lo + 8], 16)
        lens[:B, 1] = lengths

        in_maps.append({"bigf": bigf, "wband": wband, "masks2": masks2,
                        "lens": lens})
    return in_maps


def combine_partials(partials):
    """partials: list of 8 arrays [8,1] -> final scalar (0-d f32 ndarray)."""
    ps = np.stack([np.asarray(p, dtype=np.float64).reshape(8) for p in partials])
    dc_w = ps[:, 0].sum()
    mel_num = ps[:, 1].sum() + ps[:, 2].sum() - ps[:, 3].sum()
    logp = ps[:, 5].sum()
    mask_cnt = ps[:, 4].sum()
    len_sum = ps[0, 6]
    mel_loss = mel_num / float(B * T * NMEL)
    stop_loss = -5.0 * logp / mask_cnt
    dc = dc_w / (H * len_sum * N)
    return np.array(np.float32(mel_loss + stop_loss - 1e-4 * dc))


def kernel(lengths, mask, stop_pred, mels_pred, mels_target, alignments):
    from concourse.bass_utils import run_bass_kernel_spmd

    nc = _get_nc()
    in_maps = make_in_maps(lengths, np.asarray(mask), stop_pred,
                           mels_pred, mels_target, alignments)
    res = run_bass_kernel_spmd(nc, in_maps, list(range(NCORES)))
    return combine_partials([r["out"] for r in res.results])


# revision 13
# speedup vs baseline: 1.4703x; 1.3262x over previous
"""Trainium2 Bass kernel for nn_Loss_20993800143146 (loss_fn).

Computes, over 8 NeuronCores (data-parallel over batch / bh):
    mel_loss  = mean(|mels_pred * mask - mels_target|)           (mean over full tensor)
    stop_loss = sum(-5 * clamp(log(stop_pred[b, last_idx_b]), -100)) / mask.sum()
    dc        = sum(alignments * band[s,t] * bmask[b]) / (H * lengths.sum() * N)
    out       = mel_loss + stop_loss - 1e-4 * dc

Key algebraic fact: band[s,t] = (s >= clip(5t-50,0,160)) & (s < clip(5t+50,0,160))
is identically zero for t >= 42 (clip hits s=160), so only alignments[:,:,:,:42]
is ever read (~5 MB of the 98 MB tensor).

Sharding: batch dim (16 -> 2 per core) for lengths/mask/stop/mels, bh dim
(64 -> 8 per core) for alignments. Each core reduces its shard to 8 partial
scalars on-device; the host sums the 8 partial vectors and applies the final
constant-denominator arithmetic.

Per-core layouts (self-contained; all shapes hardcoded):
  bigf [128, 3366] f32 = stop13S(13) | iota13S(13) | melsp(1040) | melst(1040)
                         | align(1260), DMA'd in chunks on the SP sequencer.
  - melsp/melst: [2,800,80] -> rows (b,t) padded 1600->1664 -> 13 rows of 80
    per partition (contiguous per partition).
  - align: alignments[:, 8c:8c+8, :, :42] transposed to [b_local, n, s, t],
    16 partitions per b_local, 30 (n,s)-rows of 42 per partition (contiguous).
  - stop13S/iota13S: stop_pred split per batch row: b0 -> partitions 0..63,
    b1 -> partitions 64..127, 13 t's per partition; iota holds t+1 (0 = pad,
    stop pad = 1.0 so Ln is finite).
  wband [128,1260] u8: band weight per align element (cast to f32 on ACT).
  masks2 [128,26] u8: mask in mel layout (cols 0:13) and stop layout (13:26).
  lens [128,2] i32: col0 = lengths[b_local(p)] (bmask), col1 = lengths (p<16).
  ident [128,128] f32: identity for PE transposes (stop cross-partition max).

Stats tile [128,8] reduced across partitions with one PE matmul against ones:
  cols: 0=dc_w, 1=melA(sum m|d|), 2=melB(sum|b|), 3=melC(sum m|b|),
        4=mask_cnt, 5=logp_b0, 6=lengths_sum, 7=logp_b1.
"""

import numpy as np

# Problem constants (hardcoded per contract; kernel.py must be self-contained).
H = 4
B = 16
T = 800
NMEL = 80
S = 160
N = 3
BW = 50
K = T // S  # 5
TC = 42  # band[:, t] == 0 for all t >= TC
NCORES = 8

MEL_ROWS = 2 * T            # 1600 (b,t) rows per core
MEL_PAD_ROWS = 1664         # pad to 128 * 13
MG = 13                     # 80-col groups per partition (mel) / t's (stop)
ALN_F = N * S * TC // 16    # 1260 free elems per partition (8 b * 16 part/b)
BIGF = 2 * MG + 2 * MG * NMEL + ALN_F  # 3366

_CACHE = {}


def _band():
    tr = np.arange(TC)
    mn = np.clip(K * tr - BW, 0, S)
    mx = np.clip(K * tr + BW, 0, S)
    rows = np.arange(S)
    return ((rows[:, None] >= mn[None, :]) & (rows[:, None] < mx[None, :]))


def _wband_u8():
    """Band weight tile [128, 1260]: partition p holds rows (p%16)*30+j of the
    (n, s) x t[:TC] block of one b; weight depends only on s = row % 160."""
    band = _band()  # [S, TC] bool
    p_idx = np.arange(128)
    j_idx = np.arange(30)
    s_of = (((p_idx[:, None] % 16) * 30) + j_idx[None, :]) % S  # [128, 30]
    return band[s_of].reshape(128, ALN_F).astype(np.uint8)


def _iota13s():
    """[128,13] f32: t+1 in the stop split layout, 0 in pad positions."""
    out = np.zeros((128, MG), np.float32)
    for p in range(128):
        base = 13 * (p % 64)
        for j in range(MG):
            t = base + j
            if t < T:
                out[p, j] = t + 1
    return out


def _split13(row, pad_value):
    """[800] -> [64,13] padded with pad_value."""
    out = np.full((64 * MG,), pad_value, row.dtype)
    out[:T] = row
    return out.reshape(64, MG)


def _build_bass():
    import concourse.bacc as bacc
    import concourse.tile as tile
    import concourse.mybir as mybir
    from contextlib import ExitStack

    f32 = mybir.dt.float32
    u8 = mybir.dt.uint8
    i32 = mybir.dt.int32
    Alu = mybir.AluOpType
    Act = mybir.ActivationFunctionType
    Ax = mybir.AxisListType

    nc = bacc.Bacc("TRN2", target_bir_lowering=False, debug=False,
                   num_devices=NCORES)

    bigf = nc.dram_tensor("bigf", [128, BIGF], f32, kind="ExternalInput").ap()
    wband = nc.dram_tensor("wband", [128, ALN_F], u8, kind="ExternalInput").ap()
    masks2 = nc.dram_tensor("masks2", [128, 2 * MG], u8, kind="ExternalInput").ap()
    lens = nc.dram_tensor("lens", [128, 2], i32, kind="ExternalInput").ap()
    ident = nc.dram_tensor("ident", [128, 128], f32, kind="ExternalInput").ap()
    out = nc.dram_tensor("out", [8, 1], f32, kind="ExternalOutput").ap()

    C_STOP, C_IOTA = 0, MG
    C_MP = 2 * MG
    C_MT = C_MP + MG * NMEL
    C_AL = C_MT + MG * NMEL
    AL_HALF = ALN_F // 2  # 630

    with tile.TileContext(nc) as tc:
        with ExitStack() as ctx:
            pool = ctx.enter_context(tc.tile_pool(name="main", bufs=1))
            ppool = ctx.enter_context(tc.tile_pool(name="ps", bufs=1, space="PSUM"))

            big_t = pool.tile([128, BIGF], f32, tag="big")
            wf_t = pool.tile([128, ALN_F], f32, tag="wf")
            mk_t = pool.tile([128, 2 * MG], u8, tag="mk")
            ln_t = pool.tile([128, 2], i32, tag="ln")
            wb_t = pool.tile([128, ALN_F], u8, tag="wb")
            id_t = pool.tile([128, 128], f32, tag="id")

            # ---- DMA issues, all on the SP sequencer, in arrival-need order
            nc.sync.dma_start(big_t[:, 0:C_MP], bigf[:, 0:C_MP])
            nc.sync.dma_start(mk_t[:], masks2)
            nc.sync.dma_start(big_t[:, C_MT:C_AL], bigf[:, C_MT:C_AL])
            nc.sync.dma_start(big_t[:, C_MP:C_MT], bigf[:, C_MP:C_MT])
            nc.sync.dma_start(wb_t[:], wband)
            nc.sync.dma_start(big_t[:, C_AL:C_AL + AL_HALF],
                              bigf[:, C_AL:C_AL + AL_HALF])
            nc.sync.dma_start(big_t[:, C_AL + AL_HALF:BIGF],
                              bigf[:, C_AL + AL_HALF:BIGF])
            nc.sync.dma_start(ln_t[:], lens)
            nc.sync.dma_start(id_t[:], ident)

            # stats[:, c]: 0=dc_w, 1=melA, 2=melB, 3=melC, 4=mask_cnt,
            # 5=logp_b0, 6=len_sum, 7=logp_b1
            st_t = pool.tile([128, 8], f32, tag="st")
            nc.vector.memset(st_t[:], 0.0)
            on_t = pool.tile([128, 1], f32, tag="on")
            nc.vector.memset(on_t[:], 1.0)

            stop_v = big_t[:, C_STOP:C_STOP + MG]
            iota_v = big_t[:, C_IOTA:C_IOTA + MG]
            mp_v = big_t[:, C_MP:C_MT].rearrange("p (g m) -> p g m", m=NMEL)
            mt_v = big_t[:, C_MT:C_AL].rearrange("p (g m) -> p g m", m=NMEL)

            # band-weight u8 -> f32 cast on the scalar engine
            nc.scalar.activation(wf_t[:], wb_t[:], Act.Copy)

            # ---- stop term stage A (b0 on partitions 0:64, b1 on 64:128) ----
            lp_t = pool.tile([128, MG], f32, tag="lp")
            nc.scalar.activation(lp_t[:], stop_v, Act.Ln)
            cl_t = pool.tile([128, MG], f32, tag="cl")
            nc.vector.tensor_scalar_max(cl_t[:], lp_t[:], -100.0)
            msf_t = pool.tile([128, MG], f32, tag="msf")
            nc.vector.tensor_copy(msf_t[:], mk_t[:, MG:2 * MG])
            m13f_t = pool.tile([128, MG], f32, tag="m13f")
            nc.vector.tensor_copy(m13f_t[:], mk_t[:, 0:MG])
            tl_t = pool.tile([128, MG], f32, tag="tl")
            nc.vector.tensor_mul(tl_t[:], iota_v, msf_t[:])
            mxp_t = pool.tile([128, 1], f32, tag="mxp")
            nc.vector.tensor_reduce(mxp_t[:], tl_t[:], axis=Ax.X, op=Alu.max)
            eqj_t = pool.tile([128, MG], f32, tag="eqj")
            cp_t = pool.tile([128, 1], f32, tag="cp")
            nc.vector.scalar_tensor_tensor(
                eqj_t[:], tl_t[:], mxp_t[:, 0:1], cl_t[:],
                op0=Alu.is_equal, op1=Alu.mult, accum_out=cp_t[:])
            nc.vector.tensor_reduce(st_t[:, 4:5], m13f_t[:], axis=Ax.X, op=Alu.add)

            # ---- mel term ----
            v2_t = pool.tile([128, MG], f32, tag="v2")
            nc.vector.tensor_reduce(v2_t[:], mt_v, axis=Ax.X, op=Alu.add,
                                    apply_absolute_value=True)
            d_t = pool.tile([128, MG * NMEL], f32, tag="d")
            nc.vector.tensor_sub(d_t[:], mp_v, mt_v)
            v1_t = pool.tile([128, MG], f32, tag="v1")
            nc.vector.tensor_reduce(
                v1_t[:], d_t[:].rearrange("p (g m) -> p g m", m=NMEL),
                axis=Ax.X, op=Alu.add, apply_absolute_value=True)
            w1_t = pool.tile([128, MG], f32, tag="w1")
            nc.vector.scalar_tensor_tensor(
                w1_t[:], v1_t[:], 1.0, m13f_t[:],
                op0=Alu.bypass, op1=Alu.mult, accum_out=st_t[:, 1:2])
            nc.vector.tensor_reduce(st_t[:, 2:3], v2_t[:], axis=Ax.X, op=Alu.add)
            w2_t = pool.tile([128, MG], f32, tag="w2")
            nc.vector.scalar_tensor_tensor(
                w2_t[:], v2_t[:], 1.0, m13f_t[:],
                op0=Alu.bypass, op1=Alu.mult, accum_out=st_t[:, 3:4])

            # ---- lengths ----
            lrf_t = pool.tile([128, 1], f32, tag="lrf")
            nc.vector.tensor_copy(lrf_t[:], ln_t[:, 0:1])
            nc.vector.tensor_copy(st_t[:, 6:7], ln_t[:, 1:2])
            bm_t = pool.tile([128, 1], f32, tag="bm")
            nc.vector.tensor_scalar(bm_t[:], lrf_t[:], float(T), None, op0=Alu.is_le)

            # ---- dc term (two halves so compute overlaps the 2nd DMA) ----
            al_v = big_t[:, C_AL:BIGF]
            pra_t = pool.tile([128, AL_HALF], f32, tag="pra")
            dca_t = pool.tile([128, 1], f32, tag="dca")
            nc.vector.scalar_tensor_tensor(
                pra_t[:], al_v[:, 0:AL_HALF], 1.0, wf_t[:, 0:AL_HALF],
                op0=Alu.bypass, op1=Alu.mult, accum_out=dca_t[:])
            prb_t = pool.tile([128, AL_HALF], f32, tag="prb")
            dcb_t = pool.tile([128, 1], f32, tag="dcb")
            nc.vector.scalar_tensor_tensor(
                prb_t[:], al_v[:, AL_HALF:ALN_F], 1.0, wf_t[:, AL_HALF:ALN_F],
                op0=Alu.bypass, op1=Alu.mult, accum_out=dcb_t[:])
            dcs_t = pool.tile([128, 1], f32, tag="dcs")
            nc.vector.tensor_add(dcs_t[:], dca_t[:], dcb_t[:])
            nc.vector.tensor_mul(st_t[:, 0:1], dcs_t[:], bm_t[:])

            # ---- stop stage B: transpose Mp and cp into the free dim on PE,
            # then per-b max + select on partition 0 only.
            psA = ppool.tile([1, 128], f32, tag="psA")
            nc.tensor.transpose(psA[:], mxp_t[:], id_t[:])
            psB = ppool.tile([1, 128], f32, tag="psB")
            nc.tensor.transpose(psB[:], cp_t[:], id_t[:])
            sbA_t = pool.tile([1, 128], f32, tag="sbA")
            nc.vector.tensor_copy(sbA_t[:], psA[:])
            mb0_t = pool.tile([1, 1], f32, tag="mb0")
            nc.vector.tensor_reduce(mb0_t[:], sbA_t[0:1, 0:64], axis=Ax.X, op=Alu.max)
            mb1_t = pool.tile([1, 1], f32, tag="mb1")
            nc.vector.tensor_reduce(mb1_t[:], sbA_t[0:1, 64:128], axis=Ax.X, op=Alu.max)
            ej0_t = pool.tile([1, 64], f32, tag="ej0")
            nc.vector.scalar_tensor_tensor(
                ej0_t[:], sbA_t[0:1, 0:64], mb0_t[:, 0:1], psB[0:1, 0:64],
                op0=Alu.is_equal, op1=Alu.mult, accum_out=st_t[0:1, 5:6])
            ej1_t = pool.tile([1, 64], f32, tag="ej1")
            nc.vector.scalar_tensor_tensor(
                ej1_t[:], sbA_t[0:1, 64:128], mb1_t[:, 0:1], psB[0:1, 64:128],
                op0=Alu.is_equal, op1=Alu.mult, accum_out=st_t[0:1, 7:8])

            # ---- partition reduction via PE: out[8,1] = stats.T @ ones ----
            pt = ppool.tile([8, 1], f32, tag="pt")
            nc.tensor.matmul(pt[:], lhsT=st_t[:], rhs=on_t[:],
                             start=True, stop=True)
            ex_t = pool.tile([8, 1], f32, tag="ex")
            nc.vector.tensor_copy(ex_t[:], pt[:])
            nc.sync.dma_start(out, ex_t[:])

    nc.compile()
    return nc


def _get_nc():
    if "nc" not in _CACHE:
        _CACHE["nc"] = _build_bass()
    return _CACHE["nc"]


def make_in_maps(lengths, mask, stop_pred, mels_pred, mels_target, alignments):
    """Shard full inputs into the 8 per-core input dicts."""
    lengths = np.ascontiguousarray(lengths, dtype=np.int32)
    mask_u8 = np.ascontiguousarray(mask).view(np.uint8) if mask.dtype == np.bool_ \
        else np.ascontiguousarray(mask.astype(np.uint8))
    stop_pred = np.ascontiguousarray(stop_pred, dtype=np.float32)
    mels_pred = np.ascontiguousarray(mels_pred, dtype=np.float32)
    mels_target = np.ascontiguousarray(mels_target, dtype=np.float32)
    alignments = np.ascontiguousarray(alignments, dtype=np.float32)

    wband = _wband_u8()
    iota13s = _iota13s()
    ident = np.eye(128, dtype=np.float32)

    def pad_rows(x2d, cols):
        padded = np.zeros((MEL_PAD_ROWS, cols), x2d.dtype)
        padded[:MEL_ROWS] = x2d
        return padded

    in_maps = []
    for c in range(NCORES):
        bs = slice(2 * c, 2 * c + 2)
        bigf = np.empty((128, BIGF), np.float32)
        bigf[:, 0:MG] = np.concatenate(
            [_split13(stop_pred[2 * c], np.float32(1.0)),
             _split13(stop_pred[2 * c + 1], np.float32(1.0))])
        bigf[:, MG:2 * MG] = iota13s
        bigf[:, 2 * MG:2 * MG + MG * NMEL] = \
            pad_rows(mels_pred[bs].reshape(MEL_ROWS, NMEL), NMEL).reshape(128, MG * NMEL)
        bigf[:, 2 * MG + MG * NMEL:2 * MG + 2 * MG * NMEL] = \
            pad_rows(mels_target[bs].reshape(MEL_ROWS, NMEL), NMEL).reshape(128, MG * NMEL)
        bigf[:, 2 * MG + 2 * MG * NMEL:] = np.ascontiguousarray(
            alignments[:, 8 * c:8 * c + 8, :, :TC].transpose(1, 0, 2, 3)
        ).reshape(128, ALN_F)

        masks2 = np.zeros((128, 2 * MG), np.uint8)
        masks2[:, 0:MG] = pad_rows(mask_u8[bs].reshape(MEL_ROWS, 1), 1).reshape(128, MG)
        masks2[:, MG:2 * MG] = np.concatenate(
            [_split13(mask_u8[2 * c], np.uint8(0)),
             _split13(mask_u8[2 * c + 1], np.uint8(0))])

        b_lo = 8 * (c % 2)
        lens = np.zeros((128, 2), np.int32)
        lens[:, 0] = np.repeat(lengths[b_lo:b_lo + 8], 16)
        lens[:B, 1] = lengths

        in_maps.append({"bigf": bigf, "wband": wband, "masks2": masks2,
                        "lens": lens, "ident": ident})
    return in_maps


def combine_partials(partials):
    """partials: list of 8 arrays [8,1] -> final scalar (0-d f32 ndarray)."""
    ps = np.stack([np.asarray(p, dtype=np.float64).reshape(8) for p in partials])
    dc_w = ps[:, 0].sum()
    mel_num = ps[:, 1].sum() + ps[:, 2].sum() - ps[:, 3].sum()
    logp = ps[:, 5].sum() + ps[:, 7].sum()
    mask_cnt = ps[:, 4].sum()
    len_sum = ps[0, 6]
    mel_loss = mel_num / float(B * T * NMEL)
    stop_loss = -5.0 * logp / mask_cnt
    dc = dc_w / (H * len_sum * N)
    return np.array(np.float32(mel_loss + stop_loss - 1e-4 * dc))


def kernel(lengths, mask, stop_pred, mels_pred, mels_target, alignments):
    from concourse.bass_utils import run_bass_kernel_spmd

    nc = _get_nc()
    in_maps = make_in_maps(lengths, np.asarray(mask), stop_pred,
                           mels_pred, mels_target, alignments)
    res = run_bass_kernel_spmd(nc, in_maps, list(range(NCORES)))
    return combine_partials([r["out"] for r in res.results])
